# revision 28
# baseline (speedup 1.0000x reference)
"""Trainium2 Bass kernel for nn_GCNNet_28913719837235 (5x ResGatedGraphConv + BN + global_add_pool).

Strategy (8 NeuronCores, SPMD):
  - Nodes sharded into 8 contiguous ranges of 1250; edges sharded by dst node.
  - Edges sorted by dst, grouped into 128-node windows, padded to 128-edge tiles.
  - Per layer: data-parallel matmuls k/q/v/s (f16 operands, fp32 PSUM), AllGather
    of the packed q|v table (f16), dma_gather of q|v rows by src from HBM,
    per-edge gate/msg on ACT/DVE, scatter-add via host-precomputed one-hot
    matmuls on the tensor engine, BN stats via ones-matmul + AllReduce.
  - Final layer: raw pool via one-hot matmul; BN folded into the pooled sums on
    the host (exact, since pooling is linear).
"""
import numpy as np

# problem constants (hardcoded per harness contract)
N = 10000
EDGES = 160000
G = 64
C = 8
NC = N // C          # 1250 nodes per core
WIN = 128
NW = (NC + WIN - 1) // WIN   # 10 windows per core
DIMS = [(128, 512), (512, 512), (512, 128), (128, 128), (128, 128)]
EPS = 1e-5
CHUNK = 6            # tiles per dma_gather chunk

_CACHE = {}

# debug knobs (bisection aids; full kernel = defaults)
import os
DBG_LAYERS = int(os.environ.get("GNN_DBG_LAYERS", "5"))
DBG_EDGE = os.environ.get("GNN_DBG_EDGE", "1") == "1"
DBG_BN = os.environ.get("GNN_DBG_BN", "1") == "1"
DBG_AG = os.environ.get("GNN_DBG_AG", "1") == "1"
DBG_BBC = os.environ.get("GNN_DBG_BBC", "1") == "1"
DBG_MM = os.environ.get("GNN_DBG_MM", "1") == "1"
DBG_DUMP = os.environ.get("GNN_DBG_DUMP", "")          # qv|k|r
DBG_DUMP_LAYER = int(os.environ.get("GNN_DBG_DUMP_LAYER", "0"))


def _preprocess(edge_index):
    """dst-sorted edge shards -> per-(core,window) padded tiles + one-hot mats."""
    src = np.asarray(edge_index[0], dtype=np.int64)
    dst = np.asarray(edge_index[1], dtype=np.int64)
    order = np.argsort(dst, kind="stable")
    src, dst = src[order], dst[order]

    lists = []
    for c in range(C):
        lo, hi = c * NC, (c + 1) * NC
        m = (dst >= lo) & (dst < hi)
        s_c, d_c = src[m], dst[m] - lo
        per_w = []
        for w in range(NW):
            wm = (d_c >= w * WIN) & (d_c < (w + 1) * WIN)
            per_w.append((s_c[wm], d_c[wm] - w * WIN))
        lists.append(per_w)

    T = [max((len(lists[c][w][0]) + 127) // 128 for c in range(C)) for w in range(NW)]
    chunks = []
    for w in range(NW):
        rem, ch = T[w], []
        while rem > 0:
            ch.append(min(CHUNK, rem))
            rem -= ch[-1]
        chunks.append(ch)

    cores = []
    for c in range(C):
        idx_cols, sb_tiles = [], []
        for w in range(NW):
            s_w, doff = lists[c][w]
            n = len(s_w)
            npad = T[w] * 128
            s_pad = np.zeros(npad, np.int16)
            s_pad[:n] = s_w.astype(np.int16)
            S = np.zeros((T[w], 128, WIN), np.float16)
            e = np.arange(n)
            S[e // 128, e % 128, doff] = 1.0
            for t in range(T[w]):
                sb_tiles.append(S[t])                      # S: [128e, 128n]
                sb_tiles.append(S[t].T.copy())             # B: [128n, 128e]
            t0 = 0
            for ct in chunks[w]:
                ids = s_pad[t0 * 128:(t0 + ct) * 128]
                blk = ids.reshape(-1, 16).T                # [16, ct*8]
                idx_cols.append(np.tile(blk, (8, 1)))      # replicate to 128 parts
                t0 += ct
        idx_arr = np.concatenate(idx_cols, axis=1)         # [128, ICOLS]
        sb_arr = np.concatenate(
            [t.astype(np.float16) for t in sb_tiles], axis=1)  # [128, NT*256]
        cores.append((idx_arr, sb_arr))
    return T, chunks, cores


def _build_program(T, chunks):
    import sys
    if "/opt/trn_rl_repo" not in sys.path:
        sys.path.insert(0, "/opt/trn_rl_repo")
    import concourse.bacc as bacc
    import concourse.tile as tile
    import concourse.mybir as mybir
    from concourse import library_config

    F32, F16, I16 = mybir.dt.float32, mybir.dt.float16, mybir.dt.int16
    AF = mybir.ActivationFunctionType
    OP = mybir.AluOpType
    core_ids = list(range(C))

    NT = sum(T)
    ICOLS = sum(ct * 8 for ch in chunks for ct in ch)

    nc = bacc.Bacc(None, target_bir_lowering=False)

    # ---- I/O -------------------------------------------------------------
    xT0_d = nc.declare_dram_parameter("xT0", [128, NC], F16, isOutput=False)
    idx_d = nc.declare_dram_parameter("idx", [128, ICOLS], I16, isOutput=False)
    sb_d = nc.declare_dram_parameter("sb", [128, NT * 256], F16, isOutput=False)
    pool_d = nc.declare_dram_parameter("poolm", [128, NW * G], F16, isOutput=False)
    id16_d = nc.declare_dram_parameter("id16", [128, 128], F16, isOutput=False)
    ones_d = nc.declare_dram_parameter("ones", [128, 1], F16, isOutput=False)
    w_d, b_d, gT_d, beT_d = [], [], [], []
    for l, (di, do) in enumerate(DIMS):
        kt, ktn = di // 128, do // 128
        w_d.append([nc.declare_dram_parameter(f"w{l}_{nm}", [128, kt * do], F16,
                                              isOutput=False)
                    for nm in ("q", "v", "k", "s")])
        b_d.append(nc.declare_dram_parameter(f"b{l}", [1, do], F32, isOutput=False))
        if l < 4:
            gT_d.append(nc.declare_dram_parameter(f"gT{l}", [128, ktn], F32,
                                                  isOutput=False))
            beT_d.append(nc.declare_dram_parameter(f"beT{l}", [128, ktn], F32,
                                                   isOutput=False))
    pool_out = nc.declare_dram_parameter("pool_out", [G, 128], F32, isOutput=True)
    stats_out = nc.declare_dram_parameter("stats_out", [128, 2], F32, isOutput=True)
    dbg_out = nc.declare_dram_parameter("dbg_out", [128, NW * 1024], F16,
                                        isOutput=True) if DBG_DUMP else None

    qvsh, qvfull, bnp, bnf = [], [], [], []
    for l, (di, do) in enumerate(DIMS):
        qvsh.append(nc.dram_tensor(f"qvsh{l}", [NC, 2 * do], F16))
        qvfull.append(nc.dram_tensor(f"qvfull{l}", [N, 2 * do], F16,
                                     addr_space="Shared"))
        if l < 4:
            ktn = do // 128
            bnp.append(nc.dram_tensor(f"bnp{l}", [128, 2 * ktn], F32))
            bnf.append(nc.dram_tensor(f"bnf{l}", [128, 2 * ktn], F32,
                                      addr_space="Shared"))

    with tile.TileContext(nc) as tc:
        with (
            tc.tile_pool(name="const", bufs=1) as const,
            tc.tile_pool(name="persist", bufs=1) as persist,
            tc.tile_pool(name="wpool", bufs=2) as wpool,
            tc.tile_pool(name="stage", bufs=3) as stage,
            tc.tile_pool(name="small", bufs=2) as small,
            tc.tile_pool(name="gpool", bufs=2) as gpool,
            tc.tile_pool(name="sbp", bufs=2) as sbp,
            tc.tile_pool(name="idxp", bufs=2) as idxp,
            tc.tile_pool(name="psA", bufs=3, space="PSUM") as psA,
            tc.tile_pool(name="psG", bufs=2, space="PSUM") as psG,
            tc.tile_pool(name="psS", bufs=1, space="PSUM") as psS,
            tc.tile_pool(name="psT", bufs=2, space="PSUM") as psT,
        ):
            nc.gpsimd.load_library(library_config.mlp)

            id16 = const.tile([128, 128], F16)
            nc.sync.dma_start(out=id16[:], in_=id16_d[:])
            ones = const.tile([128, 1], F16)
            nc.sync.dma_start(out=ones[:], in_=ones_d[:])
            poolm = const.tile([128, NW * G], F16)
            nc.sync.dma_start(out=poolm[:], in_=pool_d[:])

            xT_a = persist.tile([128, 4 * NC], F16)
            xT_b = persist.tile([128, 4 * NC], F16)
            kloc = persist.tile([128, NW * 512], F16)
            sloc = persist.tile([128, NW * 512], F32)
            rloc = persist.tile([128, NW * 512], F16)
            qvloc = persist.tile([128, NW * 1024], F16)

            eps_sb = const.tile([128, 1], F32)
            nc.vector.memset(eps_sb[:], EPS)

            nc.sync.dma_start(out=xT_a[:, :NC], in_=xT0_d[:])
            # zero the never-written tail rows of the last window of kloc:
            # they are multiplied by zero one-hot entries, but NaNs must not
            # reach the PE.
            tail0 = (NC - 128 * (NW - 1)) // 32 * 32   # 32-aligned partition start
            nc.vector.memset(kloc[tail0:, (NW - 1) * 512:], 0.0)

            last_stat_sb = None
            for l, (di, do) in enumerate(DIMS[:DBG_LAYERS]):
                kt, ktn = di // 128, do // 128
                xT = xT_a if l % 2 == 0 else xT_b
                xTn = xT_b if l % 2 == 0 else xT_a

                b_bc = stage.tile([128, do], F32, tag="bbc")
                if DBG_BBC:
                    nc.gpsimd.dma_start(out=b_bc[:], in_=b_d[l][:, :].to_broadcast([128, do]))
                else:
                    nc.vector.memset(b_bc[:], 0.0)

                # ---- phase A: k/q/v/s matmuls (weights-outer) ------------
                for wi in range(4):                     # q, v, k, s
                    wsb = wpool.tile([128, 4 * 512], F16, tag="w")
                    nc.sync.dma_start(out=wsb[:, :kt * do], in_=w_d[l][wi][:])
                    for m in range(NW):
                        msz = 128 if m < NW - 1 else NC - 128 * (NW - 1)
                        ps = psA.tile([128, 512], F32, tag="a")
                        if not DBG_MM:
                            nc.vector.memset(ps[:msz, :do], 0.5)
                        for j in range(kt if DBG_MM else 0):
                            nc.tensor.matmul(
                                ps[:msz, :do],
                                lhsT=xT[:, j * NC + m * 128: j * NC + m * 128 + msz],
                                rhs=wsb[:, j * do:(j + 1) * do],
                                start=(j == 0), stop=(j == kt - 1),
                                skip_group_check=True)
                        if wi == 0:
                            nc.scalar.activation(
                                out=qvloc[:msz, m * 2 * do: m * 2 * do + do],
                                in_=ps[:msz, :do], func=AF.Copy)
                        elif wi == 1:
                            nc.scalar.activation(
                                out=qvloc[:msz, m * 2 * do + do: (m + 1) * 2 * do],
                                in_=ps[:msz, :do], func=AF.Copy)
                            nc.sync.dma_start(
                                out=qvsh[l][m * 128: m * 128 + msz, :],
                                in_=qvloc[:msz, m * 2 * do: (m + 1) * 2 * do])
                        elif wi == 2:
                            nc.scalar.activation(
                                out=kloc[:msz, m * 512: m * 512 + do],
                                in_=ps[:msz, :do], func=AF.Copy)
                        else:
                            nc.vector.tensor_add(
                                out=sloc[:msz, m * 512: m * 512 + do],
                                in0=ps[:msz, :do], in1=b_bc[:msz, :])
                    if wi == 1 and DBG_AG:
                        nc.gpsimd.collective_compute(
                            "AllGather", mybir.AluOpType.bypass,
                            replica_groups=[core_ids],
                            ins=[qvsh[l][:]], outs=[qvfull[l][:]])

                if DBG_DUMP and l == DBG_DUMP_LAYER:
                    if DBG_DUMP == "qv":
                        nc.sync.dma_start(out=dbg_out[:, :], in_=qvloc[:, :])
                    elif DBG_DUMP == "k":
                        nc.sync.dma_start(out=dbg_out[:, :NW * 512], in_=kloc[:, :])
                    elif DBG_DUMP == "xt":
                        nc.sync.dma_start(out=dbg_out[:, :4 * NC], in_=xT[:, :])

                # ---- phase B: edge phase ---------------------------------
                if not DBG_EDGE:
                    continue
                stat_acc = stage.tile([128, 8], F32, tag="stacc")
                nc.vector.memset(stat_acc[:], 0.0)
                ti = 0
                for w in range(NW):
                    wsz = 128 if w < NW - 1 else NC - 128 * (NW - 1)
                    pagg = psG.tile([128, 512], F32, tag="g")
                    nt_w = T[w]
                    tw = 0
                    for ct in chunks[w]:
                        idxt = idxp.tile([128, CHUNK * 8], I16, tag="i")
                        c0 = ti * 8
                        nc.sync.dma_start(out=idxt[:, :ct * 8],
                                          in_=idx_d[:, c0:c0 + ct * 8])
                        sbt = sbp.tile([128, CHUNK * 256], F16, tag="sb")
                        nc.sync.dma_start(out=sbt[:, :ct * 256],
                                          in_=sb_d[:, ti * 256:(ti + ct) * 256])
                        qvg = gpool.tile([128, CHUNK, 2 * do], F16, tag="qv")
                        nc.gpsimd.dma_gather(
                            qvg[:, :ct, :], qvfull[l][:, :],
                            idxt[:, :ct * 8], ct * 128, ct * 128, 2 * do)
                        for t in range(ct):
                            pkq = psA.tile([128, 512], F32, tag="a")
                            nc.tensor.matmul(
                                pkq[:, :do],
                                lhsT=sbt[:, t * 256 + 128: t * 256 + 256],
                                rhs=kloc[:, w * 512: w * 512 + do],
                                start=True, stop=False, skip_group_check=True)
                            nc.tensor.matmul(
                                pkq[:, :do], lhsT=id16[:],
                                rhs=qvg[:, t, :do],
                                start=False, stop=True, skip_group_check=True)
                            gate = stage.tile([128, 512], F16, tag="gate")
                            nc.scalar.activation(out=gate[:, :do], in_=pkq[:, :do],
                                                 func=AF.Sigmoid)
                            msg = stage.tile([128, 512], F16, tag="msg")
                            nc.vector.tensor_mul(out=msg[:, :do], in0=gate[:, :do],
                                                 in1=qvg[:, t, do:2 * do])
                            nc.tensor.matmul(
                                pagg[:, :do],
                                lhsT=sbt[:, t * 256: t * 256 + 128],
                                rhs=msg[:, :do],
                                start=(tw == 0), stop=(tw == nt_w - 1),
                                skip_group_check=True)
                            tw += 1
                        ti += ct
                    z = stage.tile([128, 512], F32, tag="z")
                    nc.vector.tensor_add(out=z[:wsz, :do], in0=pagg[:wsz, :do],
                                         in1=sloc[:wsz, w * 512: w * 512 + do])
                    nc.scalar.activation(out=rloc[:wsz, w * 512: w * 512 + do],
                                         in_=z[:wsz, :do], func=AF.Relu)
                    sq = stage.tile([128, 512], F16, tag="sq")
                    nc.scalar.activation(out=sq[:wsz, :do],
                                         in_=rloc[:wsz, w * 512: w * 512 + do],
                                         func=AF.Square)
                    pstat = psS.tile([128, 8], F32, tag="st")
                    for j in range(ktn):
                        nc.tensor.matmul(
                            pstat[:, j:j + 1],
                            lhsT=rloc[:wsz, w * 512 + j * 128: w * 512 + (j + 1) * 128],
                            rhs=ones[:wsz, :], start=True, stop=True,
                            skip_group_check=True)
                        nc.tensor.matmul(
                            pstat[:, 4 + j:5 + j],
                            lhsT=sq[:wsz, j * 128:(j + 1) * 128],
                            rhs=ones[:wsz, :], start=True, stop=True,
                            skip_group_check=True)
                    nc.vector.tensor_add(out=stat_acc[:, :], in0=stat_acc[:, :],
                                         in1=pstat[:, :])

                if DBG_DUMP == "r" and l == DBG_DUMP_LAYER:
                    nc.sync.dma_start(out=dbg_out[:, :NW * 512], in_=rloc[:, :])

                stat_sb = stage.tile([128, 8], F32, tag="statsb")
                nc.vector.tensor_copy(out=stat_sb[:, :ktn], in_=stat_acc[:, :ktn])
                nc.vector.tensor_copy(out=stat_sb[:, ktn:2 * ktn],
                                      in_=stat_acc[:, 4:4 + ktn])

                if not DBG_BN:
                    continue
                if l < 4:
                    # ---- BN: all-reduce stats, apply while transposing ----
                    nc.sync.dma_start(out=bnp[l][:, :], in_=stat_sb[:, :2 * ktn])
                    nc.gpsimd.collective_compute(
                        "AllReduce", OP.add, replica_groups=[core_ids],
                        ins=[bnp[l][:]], outs=[bnf[l][:]])
                    stat_g = small.tile([128, 8], F32, tag="sg")
                    nc.sync.dma_start(out=stat_g[:, :2 * ktn], in_=bnf[l][:, :])
                    gT = small.tile([128, 4], F32, tag="gT")
                    nc.sync.dma_start(out=gT[:, :ktn], in_=gT_d[l][:])
                    beT = small.tile([128, 4], F32, tag="beT")
                    nc.sync.dma_start(out=beT[:, :ktn], in_=beT_d[l][:])
                    mean = small.tile([128, 4], F32, tag="mean")
                    nc.scalar.activation(out=mean[:, :ktn], in_=stat_g[:, :ktn],
                                         func=AF.Copy, scale=1.0 / N)
                    msq = small.tile([128, 4], F32, tag="msq")
                    nc.scalar.activation(out=msq[:, :ktn],
                                         in_=stat_g[:, ktn:2 * ktn],
                                         func=AF.Copy, scale=1.0 / N)
                    m2 = small.tile([128, 4], F32, tag="m2")
                    nc.scalar.activation(out=m2[:, :ktn], in_=mean[:, :ktn],
                                         func=AF.Square)
                    var = small.tile([128, 4], F32, tag="var")
                    nc.vector.tensor_sub(out=var[:, :ktn], in0=msq[:, :ktn],
                                         in1=m2[:, :ktn])
                    sdv = small.tile([128, 4], F32, tag="sdv")
                    nc.scalar.activation(out=sdv[:, :ktn], in_=var[:, :ktn],
                                         func=AF.Sqrt, bias=eps_sb[:, :1])
                    rstd = small.tile([128, 4], F32, tag="rstd")
                    nc.vector.reciprocal(out=rstd[:, :ktn], in_=sdv[:, :ktn])
                    scl = small.tile([128, 4], F32, tag="scl")
                    nc.vector.tensor_mul(out=scl[:, :ktn], in0=rstd[:, :ktn],
                                         in1=gT[:, :ktn])
                    tmp = small.tile([128, 4], F32, tag="tmp")
                    nc.vector.tensor_mul(out=tmp[:, :ktn], in0=mean[:, :ktn],
                                         in1=scl[:, :ktn])
                    shf = small.tile([128, 4], F32, tag="shf")
                    nc.vector.tensor_sub(out=shf[:, :ktn], in0=beT[:, :ktn],
                                         in1=tmp[:, :ktn])
                    if DBG_DUMP == "bn" and l == DBG_DUMP_LAYER:
                        for off, src in ((0, mean), (4, msq), (8, scl), (12, shf)):
                            cv = stage.tile([128, 4], F16, tag="dbgbn")
                            nc.vector.tensor_copy(out=cv[:, :ktn], in_=src[:, :ktn])
                            nc.sync.dma_start(out=dbg_out[:, off:off + ktn],
                                              in_=cv[:, :ktn])
                    for m in range(NW):
                        msz = 128 if m < NW - 1 else NC - 128 * (NW - 1)
                        for j in range(ktn):
                            pt = psT.tile([128, 128], F16, tag="t")
                            nc.tensor.transpose(
                                out=pt[:, :msz],
                                in_=rloc[:msz, m * 512 + j * 128: m * 512 + (j + 1) * 128],
                                identity=id16[:msz, :msz])
                            nc.scalar.activation(
                                out=xTn[:, j * NC + m * 128: j * NC + m * 128 + msz],
                                in_=pt[:, :msz], func=AF.Identity,
                                scale=scl[:, j:j + 1], bias=shf[:, j:j + 1])
                else:
                    # ---- final: raw pool ---------------------------------
                    ppool = psG.tile([128, 512], F32, tag="g")
                    for m in range(NW):
                        msz = 128 if m < NW - 1 else NC - 128 * (NW - 1)
                        nc.tensor.matmul(
                            ppool[:G, :128],
                            lhsT=poolm[:msz, m * G:(m + 1) * G],
                            rhs=rloc[:msz, m * 512: m * 512 + 128],
                            start=(m == 0), stop=(m == NW - 1),
                            skip_group_check=True)
                    pool_sb = stage.tile([G, 128], F32, tag="poolsb")
                    nc.vector.tensor_copy(out=pool_sb[:, :], in_=ppool[:G, :128])
                    nc.sync.dma_start(out=pool_out[:, :], in_=pool_sb[:, :])
                    nc.sync.dma_start(out=stats_out[:, :], in_=stat_sb[:, :2])

    nc.compile()
    return nc


def kernel(**inputs):
    import sys
    if "/opt/trn_rl_repo" not in sys.path:
        sys.path.insert(0, "/opt/trn_rl_repo")
    from concourse.bass_utils import run_bass_kernel_spmd

    x = np.asarray(inputs["x"], np.float32)
    edge_index = np.asarray(inputs["edge_index"])
    batch = np.asarray(inputs["batch"]).astype(np.int64)

    T, chunks, cores = _preprocess(edge_index)
    key = (tuple(T), tuple(tuple(c) for c in chunks), DBG_LAYERS, DBG_EDGE, DBG_BN,
           DBG_AG, DBG_BBC, DBG_MM, DBG_DUMP, DBG_DUMP_LAYER)
    if key not in _CACHE:
        _CACHE[key] = _build_program(T, chunks)
    nc = _CACHE[key]

    # ---- shared host arrays ---------------------------------------------
    shared = {
        "id16": np.eye(128, dtype=np.float16),
        "ones": np.ones((128, 1), np.float16),
    }
    params_host = []
    for l, (di, do) in enumerate(DIMS):
        kt, ktn = di // 128, do // 128
        Wk = np.asarray(inputs[f"p{l+1}_Wk"], np.float32)
        Wq = np.asarray(inputs[f"p{l+1}_Wq"], np.float32)
        Wv = np.asarray(inputs[f"p{l+1}_Wv"], np.float32)
        Ws = np.asarray(inputs[f"p{l+1}_Ws"], np.float32)
        b = np.asarray(inputs[f"p{l+1}_b"], np.float32)
        g = np.asarray(inputs[f"p{l+1}_g"], np.float32)
        be = np.asarray(inputs[f"p{l+1}_be"], np.float32)
        params_host.append((g, be))

        def packw(W):
            return (W.reshape(kt, 128, do).transpose(1, 0, 2)
                    .reshape(128, kt * do).astype(np.float16))
        for nm, W in (("q", Wq), ("v", Wv), ("k", Wk), ("s", Ws)):
            shared[f"w{l}_{nm}"] = packw(W)
        shared[f"b{l}"] = b.reshape(1, do)
        if l < 4:
            shared[f"gT{l}"] = g.reshape(ktn, 128).T.copy()
            shared[f"beT{l}"] = be.reshape(ktn, 128).T.copy()

    in_maps = []
    for c in range(C):
        idx_arr, sb_arr = cores[c]
        bl = batch[c * NC:(c + 1) * NC]
        poolm = np.zeros((128, NW * G), np.float16)
        for m in range(NW):
            msz = min(128, NC - m * 128)
            p = np.arange(msz)
            poolm[p, m * G + bl[m * 128: m * 128 + msz]] = 1.0
        m = dict(shared)
        m["xT0"] = x[c * NC:(c + 1) * NC, :].T.astype(np.float16).copy()
        m["idx"] = idx_arr
        m["sb"] = sb_arr
        m["poolm"] = poolm
        in_maps.append(m)

    res = run_bass_kernel_spmd(nc, in_maps, list(range(C)))
    global LAST_RES
    LAST_RES = res

    # ---- host postprocess: reduce partial pools/stats, fold final BN -----
    rawpool = np.zeros((G, 128), np.float64)
    stats = np.zeros((128, 2), np.float64)
    for c in range(C):
        rawpool += res.results[c]["pool_out"]
        stats += res.results[c]["stats_out"]
    g5, be5 = params_host[4]
    mu = stats[:, 0] / N
    var = stats[:, 1] / N - mu * mu
    scale5 = g5 / np.sqrt(var + EPS)
    shift5 = be5 - mu * scale5
    cnt = np.bincount(batch, minlength=G).astype(np.float64)
    out = rawpool * scale5[None, :] + cnt[:, None] * shift5[None, :]
    return out.astype(np.float32)
